# revision 16
# baseline (speedup 1.0000x reference)
"""Multi-head causal attention (B=4, S=1024, NX=1024, NH=16, HD=64) on 8
Trainium2 NeuronCores.

Sharding: batch (4-way) x head-group (2-way tensor parallel) = 8 cores.
Each core handles one batch element and 8 heads: it computes its slice of the
fused qkv projection, causal softmax attention for its heads, and a partial
c_proj; the host sums the two head-group partials per batch and applies the
(linear) bias corrections exactly.

Device numerics: bf16 matmul operands, fp32 PSUM accumulation, fp32 softmax
statistics (sums/reciprocals), fp32 attention output; present (k/v) returned
in bf16 and upcast on host.

The emission order interleaves the softmax-exp stream (ACT engine) with
projection/attention matmul units so the PE never stalls on exp. Softmax
denominators come from a parallel ones-matmul into a base-0 PSUM tile
(broadcast across partitions by construction), so the whole division is two
DVE ops.
"""

import numpy as np
import ml_dtypes

import concourse.bacc as bacc
import concourse.mybir as mybir
import concourse.tile as tile
from concourse.bass_utils import run_bass_kernel_spmd

B, S, NX, NH, HD = 4, 1024, 1024, 16, 64
HG = 2            # head groups (tensor parallel dimension)
HPC = NH // HG    # heads per core = 8
FPC = HPC * HD    # head features per core = 512
NP = HPC // 2     # head pairs per core = 4
P = 128
KT = NX // P      # 8 contraction tiles for the projections
SCALE = 1.0 / float(np.sqrt(HD))

F32 = mybir.dt.float32
BF16 = mybir.dt.bfloat16
Exp = mybir.ActivationFunctionType.Exp

_CACHED_NC = None
_last_in_maps = None


def _valid_j(c):
    """s_k 128-tiles needed for s_q chunk c (chunks of 512): causal."""
    return range(4) if c == 0 else range(8)


def _interleave(primary, filler, ratio):
    """Yield from primary, inserting one filler unit after every `ratio`."""
    fi = iter(filler)
    for i, u in enumerate(primary):
        yield u
        if (i + 1) % ratio == 0:
            for f in fi:
                yield f
                break
    yield from fi


def _build_nc():
    nc = bacc.Bacc(None, target_bir_lowering=False)

    xT_d = nc.dram_tensor("xT", [NX, S], BF16, kind="ExternalInput")
    wqk_d = nc.dram_tensor("w_qk", [NX, 2 * FPC], BF16, kind="ExternalInput")
    wv_d = nc.dram_tensor("w_v", [NX, FPC], BF16, kind="ExternalInput")
    wp_d = nc.dram_tensor("w_p", [FPC, NX], BF16, kind="ExternalInput")
    bqk_d = nc.dram_tensor("b_qk", [P, 8], F32, kind="ExternalInput")

    out_d = nc.dram_tensor("out_p", [S, NX], F32, kind="ExternalOutput")
    kout_d = nc.dram_tensor("k_out", [FPC, S], BF16, kind="ExternalOutput")
    vout_d = nc.dram_tensor("v_out", [S, FPC], BF16, kind="ExternalOutput")

    with tile.TileContext(nc) as tc:
        with (
            tc.tile_pool(name="persist", bufs=1) as persist,
            tc.tile_pool(name="stage", bufs=2) as stage,
        ):
            bias_sb = persist.tile([P, 8], F32, tag="bias", name="bias")
            nc.sync.dma_start(bias_sb[:], bqk_d[:])
            ones64 = persist.tile([P, HD], BF16, tag="ones64", name="ones64")
            nc.gpsimd.memset(ones64[:], 1.0)

            qT_sb = [persist.tile([P, S], BF16, tag=f"qT{m}", name=f"qT{m}")
                     for m in range(NP)]
            kT_sb = [persist.tile([P, S], BF16, tag=f"kT{m}", name=f"kT{m}")
                     for m in range(NP)]
            v_sb = [persist.tile([P, HPC, HD], BF16, tag=f"v{i}",
                                 name=f"v{i}") for i in range(KT)]
            aT_sb = [persist.tile([P, S], BF16, tag=f"aT{m}", name=f"aT{m}")
                     for m in range(NP)]
            wp_sb = persist.tile([P, NP, NX], BF16, tag="wp", name="wp")

            with (
                tc.tile_pool(name="loads", bufs=1) as loads,
                tc.tile_pool(name="att", bufs=2) as att,
                tc.tile_pool(name="small", bufs=3) as small,
                tc.tile_pool(name="ps_w", bufs=2, space="PSUM") as ps_w,
            ):
                # ---- input DMAs: 2-ktile chunks, alternating rings ---------
                xT_sb = loads.tile([P, KT, S], BF16, tag="xT", name="xT")
                wqk_sb = loads.tile([P, KT, 2 * FPC], BF16, tag="wqk",
                                    name="wqk")
                wv_sb = loads.tile([P, KT, FPC], BF16, tag="wv", name="wv")
                for c4 in range(4):
                    kks = slice(c4 * 2, c4 * 2 + 2)
                    rs = slice(c4 * 2 * P, (c4 * 2 + 2) * P)
                    nc.sync.dma_start(
                        xT_sb[:, kks, :],
                        xT_d[rs, :].rearrange("(kt p) s -> p kt s", p=P))
                    nc.scalar.dma_start(
                        wqk_sb[:, kks, :],
                        wqk_d[rs, :].rearrange("(kt p) f -> p kt f", p=P))
                    nc.sync.dma_start(
                        wv_sb[:, kks, :],
                        wv_d[rs, :].rearrange("(kt p) f -> p kt f", p=P))
                nc.scalar.dma_start(
                    wp_sb[:], wp_d[:].rearrange("(m p) f -> p m f", p=P))

                pT_tiles = {}

                # ---------------- unit emitters -----------------------------
                def qk_unit(args, ps1):
                    m, h = args  # m: 0-3 q tiles, 4-7 k tiles; h: s half
                    pu = ps1.tile([P, 512], F32, tag="ps1", name="ps1")
                    for kk in range(KT):
                        nc.tensor.matmul(
                            pu[:],
                            wqk_sb[:, kk, m * P:(m + 1) * P],
                            xT_sb[:, kk, h * 512:(h + 1) * 512],
                            start=(kk == 0), stop=(kk == KT - 1),
                        )
                    dst = qT_sb[m] if m < NP else kT_sb[m - NP]
                    nc.vector.tensor_scalar_add(
                        dst[:, h * 512:(h + 1) * 512], pu[:],
                        bias_sb[:, m:m + 1])
                    if m >= NP:  # present-k: ship the bf16 tile directly
                        nc.scalar.dma_start(
                            kout_d[(m - NP) * P:(m - NP + 1) * P,
                                   h * 512:(h + 1) * 512],
                            dst[:, h * 512:(h + 1) * 512])

                def v_unit(i, ps1):
                    pu = ps1.tile([P, FPC], F32, tag="ps1", name="ps1")
                    for kk in range(KT):
                        nc.tensor.matmul(
                            pu[:],
                            xT_sb[:, kk, i * P:(i + 1) * P],
                            wv_sb[:, kk, :],
                            start=(kk == 0), stop=(kk == KT - 1),
                        )
                    nc.vector.tensor_copy(
                        v_sb[i][:].rearrange("p h d -> p (h d)"), pu[:])
                    nc.scalar.dma_start(
                        vout_d[i * P:(i + 1) * P, :],
                        v_sb[i][:].rearrange("p h d -> p (h d)"))

                def sc_unit(args, _ps1=None):
                    p, c, j = args
                    pT = pT_tiles[p]
                    partial = j * P >= c * 512
                    y0 = j * P - c * 512 if partial else 0
                    for hs in range(2):
                        pw = ps_w.tile([P, 512], F32, tag="psw", name="psw")
                        nc.tensor.matmul(
                            pw[:, y0:512],
                            kT_sb[p][hs * 64:(hs + 1) * 64, j * P:(j + 1) * P],
                            qT_sb[p][hs * 64:(hs + 1) * 64,
                                     c * 512 + y0:(c + 1) * 512],
                            start=True, stop=True,
                        )
                        nc.scalar.activation(
                            pT[:, hs, j, c * 512 + y0:(c + 1) * 512],
                            pw[:, y0:512], Exp, scale=SCALE)
                        if partial:
                            # causal-zero the diagonal 128-col strip
                            diag = pT[:, hs, j, j * P:(j + 1) * P]
                            nc.gpsimd.affine_select(
                                out=diag, in_=diag,
                                compare_op=mybir.AluOpType.is_ge,
                                fill=0.0, base=0,
                                pattern=[[1, P]],
                                channel_multiplier=-1,
                            )

                def pv_unit(args, pools):
                    ps_a, ps_s = pools
                    p, c, hs = args
                    pT = pT_tiles[p]
                    h = 2 * p + hs
                    pa = ps_a.tile([HD, 512], F32, tag="psa", name="psa")
                    ps2 = ps_s.tile([HD, 512], F32, tag="ps2", name="ps2")
                    js = list(_valid_j(c))
                    for j in js:
                        y0 = max(0, j * P - c * 512)
                        rhs = pT[:, hs, j, c * 512 + y0:(c + 1) * 512]
                        nc.tensor.matmul(
                            pa[:, y0:512], v_sb[j][:, h, :], rhs,
                            start=(j == js[0]), stop=(j == js[-1]))
                        nc.tensor.matmul(
                            ps2[:, y0:512], ones64[:], rhs,
                            start=(j == js[0]), stop=(j == js[-1]))
                    divb = small.tile([HD, 512], F32, tag="divb", name="divb")
                    nc.vector.reciprocal_approx_fast(out=divb[:], in_=ps2[:])
                    if hs == 0:
                        nc.vector.tensor_mul(
                            out=aT_sb[p][0:HD, c * 512:(c + 1) * 512],
                            in0=pa[:], in1=divb[:])
                    else:
                        tmp = small.tile([HD, 512], BF16, tag="tmp",
                                         name="tmp")
                        nc.vector.tensor_mul(out=tmp[:], in0=pa[:],
                                             in1=divb[:])
                        nc.sync.dma_start(
                            aT_sb[p][HD:P, c * 512:(c + 1) * 512], tmp[:])

                def sc_pair(p):
                    pT_tiles[p] = att.tile([P, 2, 8, S], BF16, tag="pT",
                                           name="pT")
                    return [(p, c, j) for c in range(2) for j in _valid_j(c)]

                def pv_pair(p):
                    return [(p, c, hs) for c in range(2) for hs in range(2)]

                # ---------------- schedule ----------------------------------
                with tc.tile_pool(name="ps_1", bufs=3, space="PSUM") as ps1:
                    # HAM warmup: dummy matmuls while input DMAs stream
                    warm = small.tile([P, 512], BF16, tag="warm", name="warm")
                    nc.gpsimd.memset(warm[:], 0.0)
                    pwarm = ps1.tile([P, 512], F32, tag="ps1", name="ps1")
                    for _ in range(52):
                        nc.tensor.matmul(pwarm[:], warm[:, 0:P], warm[:],
                                         start=True, stop=True)
                    qk = [(m, h) for m in (0, 4, 1, 5, 2, 6, 3, 7)
                          for h in range(2)]
                    for u in qk[:8]:
                        qk_unit(u, ps1)
                    for kind, u in _interleave(
                            [("sc", u) for u in sc_pair(0)],
                            [("qk", u) for u in qk[8:12]], 3):
                        (sc_unit if kind == "sc" else qk_unit)(u, ps1)
                    for kind, u in _interleave(
                            [("sc", u) for u in sc_pair(1)],
                            [("qk", u) for u in qk[12:16]], 3):
                        (sc_unit if kind == "sc" else qk_unit)(u, ps1)
                    for i in range(KT):
                        v_unit(i, ps1)

                with (
                    tc.tile_pool(name="ps_a", bufs=2, space="PSUM") as ps_a,
                    tc.tile_pool(name="ps_s", bufs=2, space="PSUM") as ps_s,
                ):
                    pools = (ps_a, ps_s)
                    for kind, u in _interleave(
                            [("sc", u) for u in sc_pair(2)],
                            [("pv", u) for u in pv_pair(0)], 3):
                        (sc_unit if kind == "sc" else pv_unit)(u, pools)
                    for kind, u in _interleave(
                            [("sc", u) for u in sc_pair(3)],
                            [("pv", u) for u in pv_pair(1)], 3):
                        (sc_unit if kind == "sc" else pv_unit)(u, pools)
                    for u in pv_pair(2):
                        pv_unit(u, pools)
                    for u in pv_pair(3):
                        pv_unit(u, pools)

                # ---- c_proj -----------------------------------------------
                with tc.tile_pool(name="ps_o", bufs=4, space="PSUM") as ps_o2:
                    for i in range(KT):
                        for n in range(2):
                            po = ps_o2.tile([P, 512], F32, tag="pso",
                                            name="pso")
                            for m in range(NP):
                                nc.tensor.matmul(
                                    po[:],
                                    aT_sb[m][:, i * P:(i + 1) * P],
                                    wp_sb[:, m, n * 512:(n + 1) * 512],
                                    start=(m == 0),
                                    stop=(m == NP - 1),
                                )
                            ost = stage.tile([P, 512], F32, tag="ostage",
                                             name="ostage")
                            if (i * 2 + n) % 2 == 0:
                                nc.scalar.copy(ost[:], po[:])
                            else:
                                nc.vector.tensor_copy(ost[:], po[:])
                            nc.sync.dma_start(
                                out_d[i * P:(i + 1) * P,
                                      n * 512:(n + 1) * 512], ost[:])

    nc.finalize()
    return nc


def _get_nc():
    global _CACHED_NC
    if _CACHED_NC is None:
        _CACHED_NC = _build_nc()
    return _CACHED_NC


def kernel(x, w_attn, b_attn, w_proj, b_proj, causal_bias):
    x = np.asarray(x, dtype=np.float32)
    w_attn = np.asarray(w_attn, dtype=np.float32)
    b_attn = np.asarray(b_attn, dtype=np.float32)
    w_proj = np.asarray(w_proj, dtype=np.float32)
    b_proj = np.asarray(b_proj, dtype=np.float32)

    nc = _get_nc()

    bf = ml_dtypes.bfloat16
    xT = [np.ascontiguousarray(x[b].T).astype(bf) for b in range(B)]
    wqk = []
    wv = []
    wp = []
    bqk = []
    for g in range(HG):
        qs = slice(g * FPC, (g + 1) * FPC)
        ks = slice(NX + g * FPC, NX + (g + 1) * FPC)
        vs = slice(2 * NX + g * FPC, 2 * NX + (g + 1) * FPC)
        wqk.append(np.ascontiguousarray(
            np.concatenate([w_attn[:, qs], w_attn[:, ks]], axis=1)).astype(bf))
        wv.append(np.ascontiguousarray(w_attn[:, vs]).astype(bf))
        wp.append(np.ascontiguousarray(
            w_proj[g * FPC:(g + 1) * FPC, :]).astype(bf))
        bqk.append(np.ascontiguousarray(
            np.concatenate([b_attn[qs], b_attn[ks]]).reshape(8, P).T
        ).astype(np.float32))

    in_maps = []
    for b in range(B):
        for g in range(HG):
            in_maps.append({
                "xT": xT[b], "w_qk": wqk[g], "w_v": wv[g], "w_p": wp[g],
                "b_qk": bqk[g],
            })

    global _last_in_maps
    _last_in_maps = in_maps
    res = run_bass_kernel_spmd(nc, in_maps, core_ids=list(range(8)))

    # ---- host-side gather / unshard ------------------------------------
    out = np.empty((B, S, NX), dtype=np.float32)
    present = np.empty((2, B, NH, S, HD), dtype=np.float32)

    # exact linear corrections for the v and proj biases (device omits them)
    corr = b_proj.astype(np.float64).copy()
    for g in range(HG):
        b_v = b_attn[2 * NX + g * FPC: 2 * NX + (g + 1) * FPC]
        corr += b_v.astype(np.float64) @ w_proj[
            g * FPC:(g + 1) * FPC, :].astype(np.float64)
    corr = corr.astype(np.float32)

    for b in range(B):
        r0 = res.results[b * HG + 0]
        r1 = res.results[b * HG + 1]
        out[b] = r0["out_p"] + r1["out_p"] + corr[None, :]
        for g, r in ((0, r0), (1, r1)):
            b_v = b_attn[2 * NX + g * FPC: 2 * NX + (g + 1) * FPC]
            # k_out already includes the (q,k) bias; v bias added here
            k_full = r["k_out"].astype(np.float32)          # [FPC, S]
            v_full = r["v_out"].astype(np.float32) + b_v[None, :]  # [S, FPC]
            present[0, b, g * HPC:(g + 1) * HPC] = (
                k_full.reshape(HPC, HD, S).transpose(0, 2, 1))
            present[1, b, g * HPC:(g + 1) * HPC] = (
                v_full.reshape(S, HPC, HD).transpose(1, 0, 2))

    return out, present


# revision 17
# speedup vs baseline: 1.0114x; 1.0114x over previous
"""Multi-head causal attention (B=4, S=1024, NX=1024, NH=16, HD=64) on 8
Trainium2 NeuronCores.

Sharding: batch (4-way) x head-group (2-way tensor parallel) = 8 cores.
Each core handles one batch element and 8 heads: it computes its slice of the
fused qkv projection, causal softmax attention for its heads, and a partial
c_proj; the host sums the two head-group partials per batch and applies the
(linear) bias corrections exactly.

Device numerics: bf16 matmul operands, fp32 PSUM accumulation, fp32 softmax
statistics (sums/reciprocals), fp32 attention output; present (k/v) returned
in bf16 and upcast on host.

The emission order interleaves the softmax-exp stream (ACT engine) with
projection/attention matmul units so the PE never stalls on exp. Softmax
denominators come from a parallel ones-matmul into a base-0 PSUM tile
(broadcast across partitions by construction), so the whole division is two
DVE ops.
"""

import numpy as np
import ml_dtypes

import concourse.bacc as bacc
import concourse.mybir as mybir
import concourse.tile as tile
from concourse.bass_utils import run_bass_kernel_spmd

B, S, NX, NH, HD = 4, 1024, 1024, 16, 64
HG = 2            # head groups (tensor parallel dimension)
HPC = NH // HG    # heads per core = 8
FPC = HPC * HD    # head features per core = 512
NP = HPC // 2     # head pairs per core = 4
P = 128
KT = NX // P      # 8 contraction tiles for the projections
SCALE = 1.0 / float(np.sqrt(HD))

F32 = mybir.dt.float32
BF16 = mybir.dt.bfloat16
Exp = mybir.ActivationFunctionType.Exp

_CACHED_NC = None
_last_in_maps = None


def _valid_j(c):
    """s_k 128-tiles needed for s_q chunk c (chunks of 512): causal."""
    return range(4) if c == 0 else range(8)


def _interleave(primary, filler, ratio):
    """Yield from primary, inserting one filler unit after every `ratio`."""
    fi = iter(filler)
    for i, u in enumerate(primary):
        yield u
        if (i + 1) % ratio == 0:
            for f in fi:
                yield f
                break
    yield from fi


def _build_nc():
    nc = bacc.Bacc(None, target_bir_lowering=False)

    xT_d = nc.dram_tensor("xT", [NX, S], BF16, kind="ExternalInput")
    wqk_d = nc.dram_tensor("w_qk", [NX, 2 * FPC], BF16, kind="ExternalInput")
    wv_d = nc.dram_tensor("w_v", [NX, FPC], BF16, kind="ExternalInput")
    wp_d = nc.dram_tensor("w_p", [FPC, NX], BF16, kind="ExternalInput")
    bqk_d = nc.dram_tensor("b_qk", [P, 8], F32, kind="ExternalInput")

    out_d = nc.dram_tensor("out_p", [S, NX], F32, kind="ExternalOutput")
    kout_d = nc.dram_tensor("k_out", [FPC, S], BF16, kind="ExternalOutput")
    vout_d = nc.dram_tensor("v_out", [S, FPC], BF16, kind="ExternalOutput")

    with tile.TileContext(nc) as tc:
        with (
            tc.tile_pool(name="persist", bufs=1) as persist,
            tc.tile_pool(name="stage", bufs=2) as stage,
        ):
            bias_sb = persist.tile([P, 8], F32, tag="bias", name="bias")
            nc.sync.dma_start(bias_sb[:], bqk_d[:])
            ones64 = persist.tile([P, HD], BF16, tag="ones64", name="ones64")
            nc.gpsimd.memset(ones64[:], 1.0)

            qT_sb = [persist.tile([P, S], BF16, tag=f"qT{m}", name=f"qT{m}")
                     for m in range(NP)]
            kT_sb = [persist.tile([P, S], BF16, tag=f"kT{m}", name=f"kT{m}")
                     for m in range(NP)]
            v_sb = [persist.tile([P, HPC, HD], BF16, tag=f"v{i}",
                                 name=f"v{i}") for i in range(KT)]
            aT_sb = [persist.tile([P, S], BF16, tag=f"aT{m}", name=f"aT{m}")
                     for m in range(NP)]
            wp_sb = persist.tile([P, NP, NX], BF16, tag="wp", name="wp")

            with (
                tc.tile_pool(name="loads", bufs=1) as loads,
                tc.tile_pool(name="att", bufs=2) as att,
                tc.tile_pool(name="small", bufs=3) as small,
                tc.tile_pool(name="ps_w", bufs=2, space="PSUM") as ps_w,
            ):
                # ---- input DMAs: 2-ktile chunks, alternating rings ---------
                xT_sb = loads.tile([P, KT, S], BF16, tag="xT", name="xT")
                wqk_sb = loads.tile([P, KT, 2 * FPC], BF16, tag="wqk",
                                    name="wqk")
                wv_sb = loads.tile([P, KT, FPC], BF16, tag="wv", name="wv")
                for c4 in range(4):
                    kks = slice(c4 * 2, c4 * 2 + 2)
                    rs = slice(c4 * 2 * P, (c4 * 2 + 2) * P)
                    nc.sync.dma_start(
                        xT_sb[:, kks, :],
                        xT_d[rs, :].rearrange("(kt p) s -> p kt s", p=P))
                    nc.scalar.dma_start(
                        wqk_sb[:, kks, :],
                        wqk_d[rs, :].rearrange("(kt p) f -> p kt f", p=P))
                    nc.sync.dma_start(
                        wv_sb[:, kks, :],
                        wv_d[rs, :].rearrange("(kt p) f -> p kt f", p=P))
                nc.scalar.dma_start(
                    wp_sb[:], wp_d[:].rearrange("(m p) f -> p m f", p=P))

                pT_tiles = {}

                # ---------------- unit emitters -----------------------------
                def qk_unit(args, ps1):
                    m, h = args  # m: 0-3 q tiles, 4-7 k tiles; h: s half
                    pu = ps1.tile([P, 512], F32, tag="ps1", name="ps1")
                    for kk in range(KT):
                        nc.tensor.matmul(
                            pu[:],
                            wqk_sb[:, kk, m * P:(m + 1) * P],
                            xT_sb[:, kk, h * 512:(h + 1) * 512],
                            start=(kk == 0), stop=(kk == KT - 1),
                        )
                    dst = qT_sb[m] if m < NP else kT_sb[m - NP]
                    nc.vector.tensor_scalar_add(
                        dst[:, h * 512:(h + 1) * 512], pu[:],
                        bias_sb[:, m:m + 1])
                    if m >= NP:  # present-k: ship the bf16 tile directly
                        nc.scalar.dma_start(
                            kout_d[(m - NP) * P:(m - NP + 1) * P,
                                   h * 512:(h + 1) * 512],
                            dst[:, h * 512:(h + 1) * 512])

                def v_unit(i, ps1):
                    pu = ps1.tile([P, FPC], F32, tag="ps1", name="ps1")
                    for kk in range(KT):
                        nc.tensor.matmul(
                            pu[:],
                            xT_sb[:, kk, i * P:(i + 1) * P],
                            wv_sb[:, kk, :],
                            start=(kk == 0), stop=(kk == KT - 1),
                        )
                    nc.vector.tensor_copy(
                        v_sb[i][:].rearrange("p h d -> p (h d)"), pu[:])
                    nc.scalar.dma_start(
                        vout_d[i * P:(i + 1) * P, :],
                        v_sb[i][:].rearrange("p h d -> p (h d)"))

                def sc_unit(args, _ps1=None):
                    p, c, j = args
                    pT = pT_tiles[p]
                    partial = j * P >= c * 512
                    y0 = j * P - c * 512 if partial else 0
                    for hs in range(2):
                        pw = ps_w.tile([P, 512], F32, tag="psw", name="psw")
                        nc.tensor.matmul(
                            pw[:, y0:512],
                            kT_sb[p][hs * 64:(hs + 1) * 64, j * P:(j + 1) * P],
                            qT_sb[p][hs * 64:(hs + 1) * 64,
                                     c * 512 + y0:(c + 1) * 512],
                            start=True, stop=True,
                        )
                        nc.scalar.activation(
                            pT[:, hs, j, c * 512 + y0:(c + 1) * 512],
                            pw[:, y0:512], Exp, scale=SCALE)
                        if partial:
                            # causal-zero the diagonal 128-col strip
                            diag = pT[:, hs, j, j * P:(j + 1) * P]
                            nc.gpsimd.affine_select(
                                out=diag, in_=diag,
                                compare_op=mybir.AluOpType.is_ge,
                                fill=0.0, base=0,
                                pattern=[[1, P]],
                                channel_multiplier=-1,
                            )

                def pv_unit(args, pools):
                    ps_a, ps_s = pools
                    p, c, hs = args
                    pT = pT_tiles[p]
                    h = 2 * p + hs
                    pa = ps_a.tile([HD, 512], F32, tag="psa", name="psa")
                    ps2 = ps_s.tile([HD, 512], F32, tag="ps2", name="ps2")
                    js = list(_valid_j(c))
                    for j in js:
                        y0 = max(0, j * P - c * 512)
                        rhs = pT[:, hs, j, c * 512 + y0:(c + 1) * 512]
                        nc.tensor.matmul(
                            pa[:, y0:512], v_sb[j][:, h, :], rhs,
                            start=(j == js[0]), stop=(j == js[-1]))
                        nc.tensor.matmul(
                            ps2[:, y0:512], ones64[:], rhs,
                            start=(j == js[0]), stop=(j == js[-1]))
                    divb = small.tile([HD, 512], F32, tag="divb", name="divb")
                    nc.vector.reciprocal_approx_fast(out=divb[:], in_=ps2[:])
                    if hs == 0:
                        nc.vector.tensor_mul(
                            out=aT_sb[p][0:HD, c * 512:(c + 1) * 512],
                            in0=pa[:], in1=divb[:])
                    else:
                        tmp = small.tile([HD, 512], BF16, tag="tmp",
                                         name="tmp")
                        nc.vector.tensor_mul(out=tmp[:], in0=pa[:],
                                             in1=divb[:])
                        nc.sync.dma_start(
                            aT_sb[p][HD:P, c * 512:(c + 1) * 512], tmp[:])

                def sc_pair(p):
                    pT_tiles[p] = att.tile([P, 2, 8, S], BF16, tag="pT",
                                           name="pT")
                    return [(p, c, j) for c in range(2) for j in _valid_j(c)]

                def pv_pair(p):
                    return [(p, c, hs) for c in range(2) for hs in range(2)]

                # ---------------- schedule ----------------------------------
                with tc.tile_pool(name="ps_1", bufs=3, space="PSUM") as ps1:
                    # HAM warmup: dummy matmuls while input DMAs stream
                    warm = small.tile([P, 512], BF16, tag="warm", name="warm")
                    nc.gpsimd.memset(warm[:], 0.0)
                    pwarm = ps1.tile([P, 512], F32, tag="ps1", name="ps1")
                    for _ in range(28):
                        nc.tensor.matmul(pwarm[:], warm[:, 0:P], warm[:],
                                         start=True, stop=True)
                    qk = [(m, h) for m in (0, 4, 1, 5, 2, 6, 3, 7)
                          for h in range(2)]
                    for u in qk[:8]:
                        qk_unit(u, ps1)
                    for kind, u in _interleave(
                            [("sc", u) for u in sc_pair(0)],
                            [("qk", u) for u in qk[8:12]], 3):
                        (sc_unit if kind == "sc" else qk_unit)(u, ps1)
                    for kind, u in _interleave(
                            [("sc", u) for u in sc_pair(1)],
                            [("qk", u) for u in qk[12:16]], 3):
                        (sc_unit if kind == "sc" else qk_unit)(u, ps1)
                    for i in range(KT):
                        v_unit(i, ps1)

                with (
                    tc.tile_pool(name="ps_a", bufs=2, space="PSUM") as ps_a,
                    tc.tile_pool(name="ps_s", bufs=2, space="PSUM") as ps_s,
                ):
                    pools = (ps_a, ps_s)
                    for kind, u in _interleave(
                            [("sc", u) for u in sc_pair(2)],
                            [("pv", u) for u in pv_pair(0)], 3):
                        (sc_unit if kind == "sc" else pv_unit)(u, pools)
                    for kind, u in _interleave(
                            [("sc", u) for u in sc_pair(3)],
                            [("pv", u) for u in pv_pair(1)], 3):
                        (sc_unit if kind == "sc" else pv_unit)(u, pools)
                    for u in pv_pair(2):
                        pv_unit(u, pools)
                    for u in pv_pair(3):
                        pv_unit(u, pools)

                # ---- c_proj -----------------------------------------------
                with tc.tile_pool(name="ps_o", bufs=4, space="PSUM") as ps_o2:
                    for i in range(KT):
                        for n in range(2):
                            po = ps_o2.tile([P, 512], F32, tag="pso",
                                            name="pso")
                            for m in range(NP):
                                nc.tensor.matmul(
                                    po[:],
                                    aT_sb[m][:, i * P:(i + 1) * P],
                                    wp_sb[:, m, n * 512:(n + 1) * 512],
                                    start=(m == 0),
                                    stop=(m == NP - 1),
                                )
                            ost = stage.tile([P, 512], F32, tag="ostage",
                                             name="ostage")
                            if (i * 2 + n) % 2 == 0:
                                nc.scalar.copy(ost[:], po[:])
                            else:
                                nc.vector.tensor_copy(ost[:], po[:])
                            nc.sync.dma_start(
                                out_d[i * P:(i + 1) * P,
                                      n * 512:(n + 1) * 512], ost[:])

    nc.finalize()
    return nc


def _get_nc():
    global _CACHED_NC
    if _CACHED_NC is None:
        _CACHED_NC = _build_nc()
    return _CACHED_NC


def kernel(x, w_attn, b_attn, w_proj, b_proj, causal_bias):
    x = np.asarray(x, dtype=np.float32)
    w_attn = np.asarray(w_attn, dtype=np.float32)
    b_attn = np.asarray(b_attn, dtype=np.float32)
    w_proj = np.asarray(w_proj, dtype=np.float32)
    b_proj = np.asarray(b_proj, dtype=np.float32)

    nc = _get_nc()

    bf = ml_dtypes.bfloat16
    xT = [np.ascontiguousarray(x[b].T).astype(bf) for b in range(B)]
    wqk = []
    wv = []
    wp = []
    bqk = []
    for g in range(HG):
        qs = slice(g * FPC, (g + 1) * FPC)
        ks = slice(NX + g * FPC, NX + (g + 1) * FPC)
        vs = slice(2 * NX + g * FPC, 2 * NX + (g + 1) * FPC)
        wqk.append(np.ascontiguousarray(
            np.concatenate([w_attn[:, qs], w_attn[:, ks]], axis=1)).astype(bf))
        wv.append(np.ascontiguousarray(w_attn[:, vs]).astype(bf))
        wp.append(np.ascontiguousarray(
            w_proj[g * FPC:(g + 1) * FPC, :]).astype(bf))
        bqk.append(np.ascontiguousarray(
            np.concatenate([b_attn[qs], b_attn[ks]]).reshape(8, P).T
        ).astype(np.float32))

    in_maps = []
    for b in range(B):
        for g in range(HG):
            in_maps.append({
                "xT": xT[b], "w_qk": wqk[g], "w_v": wv[g], "w_p": wp[g],
                "b_qk": bqk[g],
            })

    global _last_in_maps
    _last_in_maps = in_maps
    res = run_bass_kernel_spmd(nc, in_maps, core_ids=list(range(8)))

    # ---- host-side gather / unshard ------------------------------------
    out = np.empty((B, S, NX), dtype=np.float32)
    present = np.empty((2, B, NH, S, HD), dtype=np.float32)

    # exact linear corrections for the v and proj biases (device omits them)
    corr = b_proj.astype(np.float64).copy()
    for g in range(HG):
        b_v = b_attn[2 * NX + g * FPC: 2 * NX + (g + 1) * FPC]
        corr += b_v.astype(np.float64) @ w_proj[
            g * FPC:(g + 1) * FPC, :].astype(np.float64)
    corr = corr.astype(np.float32)

    for b in range(B):
        r0 = res.results[b * HG + 0]
        r1 = res.results[b * HG + 1]
        out[b] = r0["out_p"] + r1["out_p"] + corr[None, :]
        for g, r in ((0, r0), (1, r1)):
            b_v = b_attn[2 * NX + g * FPC: 2 * NX + (g + 1) * FPC]
            # k_out already includes the (q,k) bias; v bias added here
            k_full = r["k_out"].astype(np.float32)          # [FPC, S]
            v_full = r["v_out"].astype(np.float32) + b_v[None, :]  # [S, FPC]
            present[0, b, g * HPC:(g + 1) * HPC] = (
                k_full.reshape(HPC, HD, S).transpose(0, 2, 1))
            present[1, b, g * HPC:(g + 1) * HPC] = (
                v_full.reshape(S, HPC, HD).transpose(1, 0, 2))

    return out, present


# revision 18
# speedup vs baseline: 1.0367x; 1.0250x over previous
"""Multi-head causal attention (B=4, S=1024, NX=1024, NH=16, HD=64) on 8
Trainium2 NeuronCores.

Sharding: batch (4-way) x head-group (2-way tensor parallel) = 8 cores.
Each core handles one batch element and 8 heads: it computes its slice of the
fused qkv projection, causal softmax attention for its heads, and a partial
c_proj; the host sums the two head-group partials per batch and applies the
(linear) bias corrections exactly.

Device numerics: bf16 matmul operands, fp32 PSUM accumulation, fp32 softmax
statistics (sums/reciprocals), fp32 attention output; present (k/v) returned
in bf16 and upcast on host.

The emission order interleaves the softmax-exp stream (ACT engine) with
projection/attention matmul units so the PE never stalls on exp. Softmax
denominators come from a parallel ones-matmul into a base-0 PSUM tile
(broadcast across partitions by construction), so the whole division is two
DVE ops.
"""

import numpy as np
import ml_dtypes

import concourse.bacc as bacc
import concourse.mybir as mybir
import concourse.tile as tile
from concourse.bass_utils import run_bass_kernel_spmd

B, S, NX, NH, HD = 4, 1024, 1024, 16, 64
HG = 2            # head groups (tensor parallel dimension)
HPC = NH // HG    # heads per core = 8
FPC = HPC * HD    # head features per core = 512
NP = HPC // 2     # head pairs per core = 4
P = 128
KT = NX // P      # 8 contraction tiles for the projections
SCALE = 1.0 / float(np.sqrt(HD))

F32 = mybir.dt.float32
BF16 = mybir.dt.bfloat16
Exp = mybir.ActivationFunctionType.Exp

_CACHED_NC = None
_last_in_maps = None


def _valid_j(c):
    """s_k 128-tiles needed for s_q chunk c (chunks of 512): causal."""
    return range(4) if c == 0 else range(8)


def _interleave(primary, filler, ratio):
    """Yield from primary, inserting one filler unit after every `ratio`."""
    fi = iter(filler)
    for i, u in enumerate(primary):
        yield u
        if (i + 1) % ratio == 0:
            for f in fi:
                yield f
                break
    yield from fi


def _build_nc():
    nc = bacc.Bacc(None, target_bir_lowering=False)

    xT_d = nc.dram_tensor("xT", [NX, S], BF16, kind="ExternalInput")
    wqk_d = nc.dram_tensor("w_qk", [NX, 2 * FPC], BF16, kind="ExternalInput")
    wv_d = nc.dram_tensor("w_v", [NX, FPC], BF16, kind="ExternalInput")
    wp_d = nc.dram_tensor("w_p", [FPC, NX], BF16, kind="ExternalInput")
    bqk_d = nc.dram_tensor("b_qk", [P, 8], F32, kind="ExternalInput")

    out_d = nc.dram_tensor("out_p", [S, NX], F32, kind="ExternalOutput")
    kout_d = nc.dram_tensor("k_out", [FPC, S], BF16, kind="ExternalOutput")
    vout_d = nc.dram_tensor("v_out", [S, FPC], BF16, kind="ExternalOutput")

    with tile.TileContext(nc) as tc:
        with (
            tc.tile_pool(name="persist", bufs=1) as persist,
            tc.tile_pool(name="stage", bufs=2) as stage,
        ):
            bias_sb = persist.tile([P, 8], F32, tag="bias", name="bias")
            nc.sync.dma_start(bias_sb[:], bqk_d[:])
            ones64 = persist.tile([P, HD], BF16, tag="ones64", name="ones64")
            nc.gpsimd.memset(ones64[:], 1.0)

            qT_sb = [persist.tile([P, S], BF16, tag=f"qT{m}", name=f"qT{m}")
                     for m in range(NP)]
            kT_sb = [persist.tile([P, S], BF16, tag=f"kT{m}", name=f"kT{m}")
                     for m in range(NP)]
            v_sb = [persist.tile([P, HPC, HD], BF16, tag=f"v{i}",
                                 name=f"v{i}") for i in range(KT)]
            aT_sb = [persist.tile([P, S], BF16, tag=f"aT{m}", name=f"aT{m}")
                     for m in range(NP)]
            wp_sb = persist.tile([P, NP, NX], BF16, tag="wp", name="wp")

            with (
                tc.tile_pool(name="loads", bufs=1) as loads,
                tc.tile_pool(name="att", bufs=2) as att,
                tc.tile_pool(name="small", bufs=3) as small,
                tc.tile_pool(name="ps_w", bufs=2, space="PSUM") as ps_w,
            ):
                # ---- input DMAs: 2-ktile chunks, alternating rings ---------
                xT_sb = loads.tile([P, KT, S], BF16, tag="xT", name="xT")
                wqk_sb = loads.tile([P, KT, 2 * FPC], BF16, tag="wqk",
                                    name="wqk")
                wv_sb = loads.tile([P, KT, FPC], BF16, tag="wv", name="wv")
                for c4 in range(4):
                    kks = slice(c4 * 2, c4 * 2 + 2)
                    rs = slice(c4 * 2 * P, (c4 * 2 + 2) * P)
                    nc.sync.dma_start(
                        xT_sb[:, kks, :],
                        xT_d[rs, :].rearrange("(kt p) s -> p kt s", p=P))
                    nc.scalar.dma_start(
                        wqk_sb[:, kks, :],
                        wqk_d[rs, :].rearrange("(kt p) f -> p kt f", p=P))
                    nc.sync.dma_start(
                        wv_sb[:, kks, :],
                        wv_d[rs, :].rearrange("(kt p) f -> p kt f", p=P))
                nc.scalar.dma_start(
                    wp_sb[:], wp_d[:].rearrange("(m p) f -> p m f", p=P))

                pT_tiles = {}

                # ---------------- unit emitters -----------------------------
                def qk_unit(args, ps1):
                    m, h = args  # m: 0-3 q tiles, 4-7 k tiles; h: s half
                    pu = ps1.tile([P, 512], F32, tag="ps1", name="ps1")
                    for kk in range(KT):
                        nc.tensor.matmul(
                            pu[:],
                            wqk_sb[:, kk, m * P:(m + 1) * P],
                            xT_sb[:, kk, h * 512:(h + 1) * 512],
                            start=(kk == 0), stop=(kk == KT - 1),
                        )
                    dst = qT_sb[m] if m < NP else kT_sb[m - NP]
                    nc.vector.tensor_scalar_add(
                        dst[:, h * 512:(h + 1) * 512], pu[:],
                        bias_sb[:, m:m + 1])
                    if m >= NP:  # present-k: ship the bf16 tile directly
                        nc.scalar.dma_start(
                            kout_d[(m - NP) * P:(m - NP + 1) * P,
                                   h * 512:(h + 1) * 512],
                            dst[:, h * 512:(h + 1) * 512])

                def v_unit(i, ps1):
                    pu = ps1.tile([P, FPC], F32, tag="ps1", name="ps1")
                    for kk in range(KT):
                        nc.tensor.matmul(
                            pu[:],
                            xT_sb[:, kk, i * P:(i + 1) * P],
                            wv_sb[:, kk, :],
                            start=(kk == 0), stop=(kk == KT - 1),
                        )
                    nc.vector.tensor_copy(
                        v_sb[i][:].rearrange("p h d -> p (h d)"), pu[:])
                    nc.scalar.dma_start(
                        vout_d[i * P:(i + 1) * P, :],
                        v_sb[i][:].rearrange("p h d -> p (h d)"))

                def sc_unit(args, _ps1=None):
                    p, c, j = args
                    pT = pT_tiles[p]
                    partial = j * P >= c * 512
                    y0 = j * P - c * 512 if partial else 0
                    for hs in range(2):
                        pw = ps_w.tile([P, 512], F32, tag="psw", name="psw")
                        nc.tensor.matmul(
                            pw[:, y0:512],
                            kT_sb[p][hs * 64:(hs + 1) * 64, j * P:(j + 1) * P],
                            qT_sb[p][hs * 64:(hs + 1) * 64,
                                     c * 512 + y0:(c + 1) * 512],
                            start=True, stop=True,
                        )
                        nc.scalar.activation(
                            pT[:, hs, j, c * 512 + y0:(c + 1) * 512],
                            pw[:, y0:512], Exp, scale=SCALE)
                        if partial:
                            # causal-zero the diagonal 128-col strip
                            diag = pT[:, hs, j, j * P:(j + 1) * P]
                            nc.gpsimd.affine_select(
                                out=diag, in_=diag,
                                compare_op=mybir.AluOpType.is_ge,
                                fill=0.0, base=0,
                                pattern=[[1, P]],
                                channel_multiplier=-1,
                            )

                def pv_unit(args, pools):
                    ps_a, ps_s = pools
                    p, c, hs = args
                    pT = pT_tiles[p]
                    h = 2 * p + hs
                    pa = ps_a.tile([HD, 512], F32, tag="psa", name="psa")
                    ps2 = ps_s.tile([HD, 512], F32, tag="ps2", name="ps2")
                    js = list(_valid_j(c))
                    for j in js:
                        y0 = max(0, j * P - c * 512)
                        rhs = pT[:, hs, j, c * 512 + y0:(c + 1) * 512]
                        nc.tensor.matmul(
                            pa[:, y0:512], v_sb[j][:, h, :], rhs,
                            start=(j == js[0]), stop=(j == js[-1]))
                        nc.tensor.matmul(
                            ps2[:, y0:512], ones64[:], rhs,
                            start=(j == js[0]), stop=(j == js[-1]))
                    divb = small.tile([HD, 512], F32, tag="divb", name="divb")
                    nc.vector.reciprocal_approx_fast(out=divb[:], in_=ps2[:])
                    if hs == 0:
                        nc.vector.tensor_mul(
                            out=aT_sb[p][0:HD, c * 512:(c + 1) * 512],
                            in0=pa[:], in1=divb[:])
                    else:
                        tmp = small.tile([HD, 512], BF16, tag="tmp",
                                         name="tmp")
                        nc.vector.tensor_mul(out=tmp[:], in0=pa[:],
                                             in1=divb[:])
                        nc.sync.dma_start(
                            aT_sb[p][HD:P, c * 512:(c + 1) * 512], tmp[:])

                def sc_pair(p):
                    pT_tiles[p] = att.tile([P, 2, 8, S], BF16, tag="pT",
                                           name="pT")
                    return [(p, c, j) for c in range(2) for j in _valid_j(c)]

                def pv_pair(p):
                    return [(p, c, hs) for c in range(2) for hs in range(2)]

                # ---------------- schedule ----------------------------------
                with tc.tile_pool(name="ps_1", bufs=3, space="PSUM") as ps1:
                    # HAM warmup: dummy matmuls while input DMAs stream
                    warm = small.tile([P, 512], BF16, tag="warm", name="warm")
                    nc.gpsimd.memset(warm[:], 0.0)
                    pwarm = ps1.tile([P, 512], F32, tag="ps1", name="ps1")
                    for _ in range(28):
                        nc.tensor.matmul(pwarm[:], warm[:, 0:P], warm[:],
                                         start=True, stop=True)
                    qk = [(m, h) for m in (0, 4, 1, 5, 2, 6, 3, 7)
                          for h in range(2)]
                    for u in qk[:8]:
                        qk_unit(u, ps1)
                    for kind, u in _interleave(
                            [("sc", u) for u in sc_pair(0)],
                            [("qk", u) for u in qk[8:12]], 3):
                        (sc_unit if kind == "sc" else qk_unit)(u, ps1)
                    for kind, u in _interleave(
                            [("sc", u) for u in sc_pair(1)],
                            [("qk", u) for u in qk[12:16]], 3):
                        (sc_unit if kind == "sc" else qk_unit)(u, ps1)
                    for i in range(KT):
                        v_unit(i, ps1)

                with (
                    tc.tile_pool(name="ps_a", bufs=2, space="PSUM") as ps_a,
                    tc.tile_pool(name="ps_s", bufs=2, space="PSUM") as ps_s,
                ):
                    pools = (ps_a, ps_s)
                    for kind, u in _interleave(
                            [("sc", u) for u in sc_pair(2)],
                            [("pv", u) for u in pv_pair(0)], 3):
                        (sc_unit if kind == "sc" else pv_unit)(u, pools)
                    for kind, u in _interleave(
                            [("sc", u) for u in sc_pair(3)],
                            [("pv", u) for u in pv_pair(1)], 3):
                        (sc_unit if kind == "sc" else pv_unit)(u, pools)
                    for u in pv_pair(2):
                        pv_unit(u, pools)
                    for u in pv_pair(3):
                        pv_unit(u, pools)

                # ---- c_proj -----------------------------------------------
                with tc.tile_pool(name="ps_o", bufs=2, space="PSUM") as ps_o2:
                    for i in range(KT):
                        for n in range(2):
                            po = ps_o2.tile([P, 512], F32, tag="pso",
                                            name="pso")
                            for m in range(NP):
                                nc.tensor.matmul(
                                    po[:],
                                    aT_sb[m][:, i * P:(i + 1) * P],
                                    wp_sb[:, m, n * 512:(n + 1) * 512],
                                    start=(m == 0),
                                    stop=(m == NP - 1),
                                )
                            ost = stage.tile([P, 512], F32, tag="ostage",
                                             name="ostage")
                            nc.vector.tensor_copy(ost[:], po[:])
                            nc.sync.dma_start(
                                out_d[i * P:(i + 1) * P,
                                      n * 512:(n + 1) * 512], ost[:])

    nc.finalize()
    return nc


def _get_nc():
    global _CACHED_NC
    if _CACHED_NC is None:
        _CACHED_NC = _build_nc()
    return _CACHED_NC


def kernel(x, w_attn, b_attn, w_proj, b_proj, causal_bias):
    x = np.asarray(x, dtype=np.float32)
    w_attn = np.asarray(w_attn, dtype=np.float32)
    b_attn = np.asarray(b_attn, dtype=np.float32)
    w_proj = np.asarray(w_proj, dtype=np.float32)
    b_proj = np.asarray(b_proj, dtype=np.float32)

    nc = _get_nc()

    bf = ml_dtypes.bfloat16
    xT = [np.ascontiguousarray(x[b].T).astype(bf) for b in range(B)]
    wqk = []
    wv = []
    wp = []
    bqk = []
    for g in range(HG):
        qs = slice(g * FPC, (g + 1) * FPC)
        ks = slice(NX + g * FPC, NX + (g + 1) * FPC)
        vs = slice(2 * NX + g * FPC, 2 * NX + (g + 1) * FPC)
        wqk.append(np.ascontiguousarray(
            np.concatenate([w_attn[:, qs], w_attn[:, ks]], axis=1)).astype(bf))
        wv.append(np.ascontiguousarray(w_attn[:, vs]).astype(bf))
        wp.append(np.ascontiguousarray(
            w_proj[g * FPC:(g + 1) * FPC, :]).astype(bf))
        bqk.append(np.ascontiguousarray(
            np.concatenate([b_attn[qs], b_attn[ks]]).reshape(8, P).T
        ).astype(np.float32))

    in_maps = []
    for b in range(B):
        for g in range(HG):
            in_maps.append({
                "xT": xT[b], "w_qk": wqk[g], "w_v": wv[g], "w_p": wp[g],
                "b_qk": bqk[g],
            })

    global _last_in_maps
    _last_in_maps = in_maps
    res = run_bass_kernel_spmd(nc, in_maps, core_ids=list(range(8)))

    # ---- host-side gather / unshard ------------------------------------
    out = np.empty((B, S, NX), dtype=np.float32)
    present = np.empty((2, B, NH, S, HD), dtype=np.float32)

    # exact linear corrections for the v and proj biases (device omits them)
    corr = b_proj.astype(np.float64).copy()
    for g in range(HG):
        b_v = b_attn[2 * NX + g * FPC: 2 * NX + (g + 1) * FPC]
        corr += b_v.astype(np.float64) @ w_proj[
            g * FPC:(g + 1) * FPC, :].astype(np.float64)
    corr = corr.astype(np.float32)

    for b in range(B):
        r0 = res.results[b * HG + 0]
        r1 = res.results[b * HG + 1]
        out[b] = r0["out_p"] + r1["out_p"] + corr[None, :]
        for g, r in ((0, r0), (1, r1)):
            b_v = b_attn[2 * NX + g * FPC: 2 * NX + (g + 1) * FPC]
            # k_out already includes the (q,k) bias; v bias added here
            k_full = r["k_out"].astype(np.float32)          # [FPC, S]
            v_full = r["v_out"].astype(np.float32) + b_v[None, :]  # [S, FPC]
            present[0, b, g * HPC:(g + 1) * HPC] = (
                k_full.reshape(HPC, HD, S).transpose(0, 2, 1))
            present[1, b, g * HPC:(g + 1) * HPC] = (
                v_full.reshape(S, HPC, HD).transpose(1, 0, 2))

    return out, present


# revision 19
# speedup vs baseline: 1.0438x; 1.0069x over previous
"""Multi-head causal attention (B=4, S=1024, NX=1024, NH=16, HD=64) on 8
Trainium2 NeuronCores.

Sharding: batch (4-way) x head-group (2-way tensor parallel) = 8 cores.
Each core handles one batch element and 8 heads: it computes its slice of the
fused qkv projection, causal softmax attention for its heads, and a partial
c_proj; the host sums the two head-group partials per batch and applies the
(linear) bias corrections exactly.

Device numerics: bf16 matmul operands, fp32 PSUM accumulation, fp32 softmax
statistics (sums/reciprocals), fp32 attention output; present (k/v) returned
in bf16 and upcast on host.

The emission order interleaves the softmax-exp stream (ACT engine) with
projection/attention matmul units so the PE never stalls on exp. Softmax
denominators come from a parallel ones-matmul into a base-0 PSUM tile
(broadcast across partitions by construction), so the whole division is two
DVE ops.
"""

import numpy as np
import ml_dtypes

import concourse.bacc as bacc
import concourse.mybir as mybir
import concourse.tile as tile
from concourse.bass_utils import run_bass_kernel_spmd

B, S, NX, NH, HD = 4, 1024, 1024, 16, 64
HG = 2            # head groups (tensor parallel dimension)
HPC = NH // HG    # heads per core = 8
FPC = HPC * HD    # head features per core = 512
NP = HPC // 2     # head pairs per core = 4
P = 128
KT = NX // P      # 8 contraction tiles for the projections
SCALE = 1.0 / float(np.sqrt(HD))

F32 = mybir.dt.float32
BF16 = mybir.dt.bfloat16
Exp = mybir.ActivationFunctionType.Exp

_CACHED_NC = None
_last_in_maps = None


def _valid_j(c):
    """s_k 128-tiles needed for s_q chunk c (chunks of 512): causal."""
    return range(4) if c == 0 else range(8)


def _interleave(primary, filler, ratio):
    """Yield from primary, inserting one filler unit after every `ratio`."""
    fi = iter(filler)
    for i, u in enumerate(primary):
        yield u
        if (i + 1) % ratio == 0:
            for f in fi:
                yield f
                break
    yield from fi


def _build_nc():
    nc = bacc.Bacc(None, target_bir_lowering=False)

    xT_d = nc.dram_tensor("xT", [NX, S], BF16, kind="ExternalInput")
    wqk_d = nc.dram_tensor("w_qk", [NX, 2 * FPC], BF16, kind="ExternalInput")
    wv_d = nc.dram_tensor("w_v", [NX, FPC], BF16, kind="ExternalInput")
    wp_d = nc.dram_tensor("w_p", [FPC, NX], BF16, kind="ExternalInput")
    bqk_d = nc.dram_tensor("b_qk", [P, 8], F32, kind="ExternalInput")

    out_d = nc.dram_tensor("out_p", [S, NX], F32, kind="ExternalOutput")
    kout_d = nc.dram_tensor("k_out", [FPC, S], BF16, kind="ExternalOutput")
    vout_d = nc.dram_tensor("v_out", [S, FPC], BF16, kind="ExternalOutput")

    with tile.TileContext(nc) as tc:
        with (
            tc.tile_pool(name="persist", bufs=1) as persist,
            tc.tile_pool(name="stage", bufs=2) as stage,
        ):
            bias_sb = persist.tile([P, 8], F32, tag="bias", name="bias")
            nc.sync.dma_start(bias_sb[:], bqk_d[:])
            ones64 = persist.tile([P, HD], BF16, tag="ones64", name="ones64")
            nc.gpsimd.memset(ones64[:], 1.0)

            qT_sb = [persist.tile([P, S], BF16, tag=f"qT{m}", name=f"qT{m}")
                     for m in range(NP)]
            kT_sb = [persist.tile([P, S], BF16, tag=f"kT{m}", name=f"kT{m}")
                     for m in range(NP)]
            v_sb = [persist.tile([P, HPC, HD], BF16, tag=f"v{i}",
                                 name=f"v{i}") for i in range(KT)]
            aT_sb = [persist.tile([P, S], BF16, tag=f"aT{m}", name=f"aT{m}")
                     for m in range(NP)]
            wp_sb = persist.tile([P, NP, NX], BF16, tag="wp", name="wp")

            with (
                tc.tile_pool(name="loads", bufs=1) as loads,
                tc.tile_pool(name="att", bufs=2) as att,
                tc.tile_pool(name="small", bufs=3) as small,
                tc.tile_pool(name="ps_w", bufs=2, space="PSUM") as ps_w,
            ):
                # ---- input DMAs: 2-ktile chunks, alternating rings ---------
                xT_sb = loads.tile([P, KT, S], BF16, tag="xT", name="xT")
                wqk_sb = loads.tile([P, KT, 2 * FPC], BF16, tag="wqk",
                                    name="wqk")
                wv_sb = loads.tile([P, KT, FPC], BF16, tag="wv", name="wv")
                for c4 in range(4):
                    kks = slice(c4 * 2, c4 * 2 + 2)
                    rs = slice(c4 * 2 * P, (c4 * 2 + 2) * P)
                    nc.sync.dma_start(
                        xT_sb[:, kks, :],
                        xT_d[rs, :].rearrange("(kt p) s -> p kt s", p=P))
                    nc.scalar.dma_start(
                        wqk_sb[:, kks, :],
                        wqk_d[rs, :].rearrange("(kt p) f -> p kt f", p=P))
                    nc.sync.dma_start(
                        wv_sb[:, kks, :],
                        wv_d[rs, :].rearrange("(kt p) f -> p kt f", p=P))
                nc.scalar.dma_start(
                    wp_sb[:], wp_d[:].rearrange("(m p) f -> p m f", p=P))

                pT_tiles = {}

                # ---------------- unit emitters -----------------------------
                def qk_unit(args, ps1):
                    m, h = args  # m: 0-3 q tiles, 4-7 k tiles; h: s half
                    pu = ps1.tile([P, 512], F32, tag="ps1", name="ps1")
                    for kk in range(KT):
                        nc.tensor.matmul(
                            pu[:],
                            wqk_sb[:, kk, m * P:(m + 1) * P],
                            xT_sb[:, kk, h * 512:(h + 1) * 512],
                            start=(kk == 0), stop=(kk == KT - 1),
                        )
                    dst = qT_sb[m] if m < NP else kT_sb[m - NP]
                    nc.vector.tensor_scalar_add(
                        dst[:, h * 512:(h + 1) * 512], pu[:],
                        bias_sb[:, m:m + 1])
                    if m >= NP:  # present-k: ship the bf16 tile directly
                        nc.scalar.dma_start(
                            kout_d[(m - NP) * P:(m - NP + 1) * P,
                                   h * 512:(h + 1) * 512],
                            dst[:, h * 512:(h + 1) * 512])

                def v_unit(i, ps1):
                    pu = ps1.tile([P, FPC], F32, tag="ps1", name="ps1")
                    for kk in range(KT):
                        nc.tensor.matmul(
                            pu[:],
                            xT_sb[:, kk, i * P:(i + 1) * P],
                            wv_sb[:, kk, :],
                            start=(kk == 0), stop=(kk == KT - 1),
                        )
                    nc.vector.tensor_copy(
                        v_sb[i][:].rearrange("p h d -> p (h d)"), pu[:])
                    nc.scalar.dma_start(
                        vout_d[i * P:(i + 1) * P, :],
                        v_sb[i][:].rearrange("p h d -> p (h d)"))

                def sc_unit(args, _ps1=None):
                    p, c, j = args
                    pT = pT_tiles[p]
                    partial = j * P >= c * 512
                    y0 = j * P - c * 512 if partial else 0
                    for hs in range(2):
                        pw = ps_w.tile([P, 512], F32, tag="psw", name="psw")
                        nc.tensor.matmul(
                            pw[:, y0:512],
                            kT_sb[p][hs * 64:(hs + 1) * 64, j * P:(j + 1) * P],
                            qT_sb[p][hs * 64:(hs + 1) * 64,
                                     c * 512 + y0:(c + 1) * 512],
                            start=True, stop=True,
                        )
                        nc.scalar.activation(
                            pT[:, hs, j, c * 512 + y0:(c + 1) * 512],
                            pw[:, y0:512], Exp, scale=SCALE)
                        if partial:
                            # causal-zero the diagonal 128-col strip
                            diag = pT[:, hs, j, j * P:(j + 1) * P]
                            nc.gpsimd.affine_select(
                                out=diag, in_=diag,
                                compare_op=mybir.AluOpType.is_ge,
                                fill=0.0, base=0,
                                pattern=[[1, P]],
                                channel_multiplier=-1,
                            )

                def pv_unit(args, pools):
                    ps_a, ps_s = pools
                    p, c, hs = args
                    pT = pT_tiles[p]
                    h = 2 * p + hs
                    pa = ps_a.tile([HD, 512], F32, tag="psa", name="psa")
                    ps2 = ps_s.tile([HD, 512], F32, tag="ps2", name="ps2")
                    js = list(_valid_j(c))
                    for j in js:
                        y0 = max(0, j * P - c * 512)
                        rhs = pT[:, hs, j, c * 512 + y0:(c + 1) * 512]
                        nc.tensor.matmul(
                            pa[:, y0:512], v_sb[j][:, h, :], rhs,
                            start=(j == js[0]), stop=(j == js[-1]))
                        nc.tensor.matmul(
                            ps2[:, y0:512], ones64[:], rhs,
                            start=(j == js[0]), stop=(j == js[-1]))
                    divb = small.tile([HD, 512], F32, tag="divb", name="divb")
                    nc.vector.reciprocal_approx_fast(out=divb[:], in_=ps2[:])
                    if hs == 0:
                        nc.vector.tensor_mul(
                            out=aT_sb[p][0:HD, c * 512:(c + 1) * 512],
                            in0=pa[:], in1=divb[:])
                    else:
                        tmp = small.tile([HD, 512], BF16, tag="tmp",
                                         name="tmp")
                        nc.vector.tensor_mul(out=tmp[:], in0=pa[:],
                                             in1=divb[:])
                        nc.sync.dma_start(
                            aT_sb[p][HD:P, c * 512:(c + 1) * 512], tmp[:])

                def sc_pair(p):
                    pT_tiles[p] = att.tile([P, 2, 8, S], BF16, tag="pT",
                                           name="pT")
                    return [(p, c, j) for c in range(2) for j in _valid_j(c)]

                def pv_pair(p):
                    return [(p, c, hs) for c in range(2) for hs in range(2)]

                # ---------------- schedule ----------------------------------
                with tc.tile_pool(name="ps_1", bufs=3, space="PSUM") as ps1:
                    # HAM warmup: dummy matmuls while input DMAs stream
                    warm = small.tile([P, 512], BF16, tag="warm", name="warm")
                    nc.gpsimd.memset(warm[:], 0.0)
                    pwarm = ps1.tile([P, 512], F32, tag="ps1", name="ps1")
                    for _ in range(28):
                        nc.tensor.matmul(pwarm[:], warm[:, 0:P], warm[:],
                                         start=True, stop=True)
                    qk = [(m, h) for m in (0, 4, 1, 5, 2, 6, 3, 7)
                          for h in range(2)]
                    for u in qk[:8]:
                        qk_unit(u, ps1)
                    for kind, u in _interleave(
                            [("sc", u) for u in sc_pair(0)],
                            [("qk", u) for u in qk[8:12]], 3):
                        (sc_unit if kind == "sc" else qk_unit)(u, ps1)
                    for kind, u in _interleave(
                            [("sc", u) for u in sc_pair(1)],
                            [("qk", u) for u in qk[12:16]], 3):
                        (sc_unit if kind == "sc" else qk_unit)(u, ps1)
                    for i in range(KT):
                        v_unit(i, ps1)

                with (
                    tc.tile_pool(name="ps_a", bufs=2, space="PSUM") as ps_a,
                    tc.tile_pool(name="ps_s", bufs=2, space="PSUM") as ps_s,
                    tc.tile_pool(name="ps_o", bufs=2, space="PSUM") as ps_o,
                ):
                    pools = (ps_a, ps_s)
                    for kind, u in _interleave(
                            [("sc", u) for u in sc_pair(2)],
                            [("pv", u) for u in pv_pair(0)], 3):
                        (sc_unit if kind == "sc" else pv_unit)(u, pools)
                    for kind, u in _interleave(
                            [("sc", u) for u in sc_pair(3)],
                            [("pv", u) for u in pv_pair(1)], 3):
                        (sc_unit if kind == "sc" else pv_unit)(u, pools)
                    for u in pv_pair(2):
                        pv_unit(u, pools)
                    for u in pv_pair(3):
                        pv_unit(u, pools)

                    # ---- c_proj -------------------------------------------
                    for i in range(KT):
                        for n in range(2):
                            po = ps_o.tile([P, 512], F32, tag="pso",
                                           name="pso")
                            for m in range(NP):
                                nc.tensor.matmul(
                                    po[:],
                                    aT_sb[m][:, i * P:(i + 1) * P],
                                    wp_sb[:, m, n * 512:(n + 1) * 512],
                                    start=(m == 0),
                                    stop=(m == NP - 1),
                                )
                            ost = stage.tile([P, 512], F32, tag="ostage",
                                             name="ostage")
                            nc.vector.tensor_copy(ost[:], po[:])
                            nc.sync.dma_start(
                                out_d[i * P:(i + 1) * P,
                                      n * 512:(n + 1) * 512], ost[:])

    nc.finalize()
    return nc


def _get_nc():
    global _CACHED_NC
    if _CACHED_NC is None:
        _CACHED_NC = _build_nc()
    return _CACHED_NC


def kernel(x, w_attn, b_attn, w_proj, b_proj, causal_bias):
    x = np.asarray(x, dtype=np.float32)
    w_attn = np.asarray(w_attn, dtype=np.float32)
    b_attn = np.asarray(b_attn, dtype=np.float32)
    w_proj = np.asarray(w_proj, dtype=np.float32)
    b_proj = np.asarray(b_proj, dtype=np.float32)

    nc = _get_nc()

    bf = ml_dtypes.bfloat16
    xT = [np.ascontiguousarray(x[b].T).astype(bf) for b in range(B)]
    wqk = []
    wv = []
    wp = []
    bqk = []
    for g in range(HG):
        qs = slice(g * FPC, (g + 1) * FPC)
        ks = slice(NX + g * FPC, NX + (g + 1) * FPC)
        vs = slice(2 * NX + g * FPC, 2 * NX + (g + 1) * FPC)
        wqk.append(np.ascontiguousarray(
            np.concatenate([w_attn[:, qs], w_attn[:, ks]], axis=1)).astype(bf))
        wv.append(np.ascontiguousarray(w_attn[:, vs]).astype(bf))
        wp.append(np.ascontiguousarray(
            w_proj[g * FPC:(g + 1) * FPC, :]).astype(bf))
        bqk.append(np.ascontiguousarray(
            np.concatenate([b_attn[qs], b_attn[ks]]).reshape(8, P).T
        ).astype(np.float32))

    in_maps = []
    for b in range(B):
        for g in range(HG):
            in_maps.append({
                "xT": xT[b], "w_qk": wqk[g], "w_v": wv[g], "w_p": wp[g],
                "b_qk": bqk[g],
            })

    global _last_in_maps
    _last_in_maps = in_maps
    res = run_bass_kernel_spmd(nc, in_maps, core_ids=list(range(8)))

    # ---- host-side gather / unshard ------------------------------------
    out = np.empty((B, S, NX), dtype=np.float32)
    present = np.empty((2, B, NH, S, HD), dtype=np.float32)

    # exact linear corrections for the v and proj biases (device omits them)
    corr = b_proj.astype(np.float64).copy()
    for g in range(HG):
        b_v = b_attn[2 * NX + g * FPC: 2 * NX + (g + 1) * FPC]
        corr += b_v.astype(np.float64) @ w_proj[
            g * FPC:(g + 1) * FPC, :].astype(np.float64)
    corr = corr.astype(np.float32)

    for b in range(B):
        r0 = res.results[b * HG + 0]
        r1 = res.results[b * HG + 1]
        out[b] = r0["out_p"] + r1["out_p"] + corr[None, :]
        for g, r in ((0, r0), (1, r1)):
            b_v = b_attn[2 * NX + g * FPC: 2 * NX + (g + 1) * FPC]
            # k_out already includes the (q,k) bias; v bias added here
            k_full = r["k_out"].astype(np.float32)          # [FPC, S]
            v_full = r["v_out"].astype(np.float32) + b_v[None, :]  # [S, FPC]
            present[0, b, g * HPC:(g + 1) * HPC] = (
                k_full.reshape(HPC, HD, S).transpose(0, 2, 1))
            present[1, b, g * HPC:(g + 1) * HPC] = (
                v_full.reshape(S, HPC, HD).transpose(1, 0, 2))

    return out, present


# revision 20
# speedup vs baseline: 1.1206x; 1.0736x over previous
"""Multi-head causal attention (B=4, S=1024, NX=1024, NH=16, HD=64) on 8
Trainium2 NeuronCores.

Sharding: batch (4-way) x head-group (2-way tensor parallel) = 8 cores.
Each core handles one batch element and 8 heads: it computes its slice of the
fused qkv projection, causal softmax attention for its heads, and a partial
c_proj; the host sums the two head-group partials per batch and applies the
(linear) bias corrections exactly.

Device numerics: bf16 matmul operands, fp32 PSUM accumulation, fp32 softmax
statistics (sums/reciprocals), fp32 attention output; present (k/v) returned
in bf16 and upcast on host.

The emission order interleaves the softmax-exp stream (ACT engine) with
projection/attention matmul units so the PE never stalls on exp. Softmax
denominators come from a parallel ones-matmul into a base-0 PSUM tile
(broadcast across partitions by construction), so the whole division is two
DVE ops.
"""

import numpy as np
import ml_dtypes

import concourse.bacc as bacc
import concourse.mybir as mybir
import concourse.tile as tile
from concourse.bass_utils import run_bass_kernel_spmd

B, S, NX, NH, HD = 4, 1024, 1024, 16, 64
HG = 2            # head groups (tensor parallel dimension)
HPC = NH // HG    # heads per core = 8
FPC = HPC * HD    # head features per core = 512
NP = HPC // 2     # head pairs per core = 4
P = 128
KT = NX // P      # 8 contraction tiles for the projections
SCALE = 1.0 / float(np.sqrt(HD))

F32 = mybir.dt.float32
BF16 = mybir.dt.bfloat16
Exp = mybir.ActivationFunctionType.Exp

_CACHED_NC = None
_last_in_maps = None


def _valid_j(c):
    """s_k 128-tiles needed for s_q chunk c (chunks of 512): causal."""
    return range(4) if c == 0 else range(8)


def _interleave(primary, filler, ratio):
    """Yield from primary, inserting one filler unit after every `ratio`."""
    fi = iter(filler)
    for i, u in enumerate(primary):
        yield u
        if (i + 1) % ratio == 0:
            for f in fi:
                yield f
                break
    yield from fi


def _build_nc():
    nc = bacc.Bacc(None, target_bir_lowering=False)

    xT_d = nc.dram_tensor("xT", [NX, S], BF16, kind="ExternalInput")
    wqk_d = nc.dram_tensor("w_qk", [NX, 2 * FPC], BF16, kind="ExternalInput")
    wv_d = nc.dram_tensor("w_v", [NX, FPC], BF16, kind="ExternalInput")
    wp_d = nc.dram_tensor("w_p", [FPC, NX], BF16, kind="ExternalInput")
    bqk_d = nc.dram_tensor("b_qk", [P, 8], F32, kind="ExternalInput")

    out_d = nc.dram_tensor("out_p", [S, NX], F32, kind="ExternalOutput")
    kout_d = nc.dram_tensor("k_out", [FPC, S], BF16, kind="ExternalOutput")
    vout_d = nc.dram_tensor("v_out", [S, FPC], BF16, kind="ExternalOutput")

    with tile.TileContext(nc) as tc:
        with (
            tc.tile_pool(name="persist", bufs=1) as persist,
            tc.tile_pool(name="stage", bufs=5) as stage,
        ):
            bias_sb = persist.tile([P, 8], F32, tag="bias", name="bias")
            nc.sync.dma_start(bias_sb[:], bqk_d[:])
            ones64 = persist.tile([P, HD], BF16, tag="ones64", name="ones64")
            nc.gpsimd.memset(ones64[:], 1.0)

            qT_sb = [persist.tile([P, S], BF16, tag=f"qT{m}", name=f"qT{m}")
                     for m in range(NP)]
            kT_sb = [persist.tile([P, S], BF16, tag=f"kT{m}", name=f"kT{m}")
                     for m in range(NP)]
            v_sb = [persist.tile([P, HPC, HD], BF16, tag=f"v{i}",
                                 name=f"v{i}") for i in range(KT)]
            aT_sb = [persist.tile([P, S], BF16, tag=f"aT{m}", name=f"aT{m}")
                     for m in range(NP)]
            wp_sb = persist.tile([P, NP, NX], BF16, tag="wp", name="wp")

            with (
                tc.tile_pool(name="loads", bufs=1) as loads,
                tc.tile_pool(name="att", bufs=2) as att,
                tc.tile_pool(name="small", bufs=3) as small,
                tc.tile_pool(name="ps_w", bufs=2, space="PSUM") as ps_w,
            ):
                # ---- input DMAs: 2-ktile chunks, alternating rings ---------
                xT_sb = loads.tile([P, KT, S], BF16, tag="xT", name="xT")
                wqk_sb = loads.tile([P, KT, 2 * FPC], BF16, tag="wqk",
                                    name="wqk")
                wv_sb = loads.tile([P, KT, FPC], BF16, tag="wv", name="wv")
                for c4 in range(4):
                    kks = slice(c4 * 2, c4 * 2 + 2)
                    rs = slice(c4 * 2 * P, (c4 * 2 + 2) * P)
                    nc.sync.dma_start(
                        xT_sb[:, kks, :],
                        xT_d[rs, :].rearrange("(kt p) s -> p kt s", p=P))
                    nc.scalar.dma_start(
                        wqk_sb[:, kks, :],
                        wqk_d[rs, :].rearrange("(kt p) f -> p kt f", p=P))
                    nc.sync.dma_start(
                        wv_sb[:, kks, :],
                        wv_d[rs, :].rearrange("(kt p) f -> p kt f", p=P))
                nc.scalar.dma_start(
                    wp_sb[:], wp_d[:].rearrange("(m p) f -> p m f", p=P))

                pT_tiles = {}

                # ---------------- unit emitters -----------------------------
                def qk_unit(args, ps1):
                    m, h = args  # m: 0-3 q tiles, 4-7 k tiles; h: s half
                    pu = ps1.tile([P, 512], F32, tag="ps1", name="ps1")
                    for kk in range(KT):
                        nc.tensor.matmul(
                            pu[:],
                            wqk_sb[:, kk, m * P:(m + 1) * P],
                            xT_sb[:, kk, h * 512:(h + 1) * 512],
                            start=(kk == 0), stop=(kk == KT - 1),
                        )
                    dst = qT_sb[m] if m < NP else kT_sb[m - NP]
                    nc.vector.tensor_scalar_add(
                        dst[:, h * 512:(h + 1) * 512], pu[:],
                        bias_sb[:, m:m + 1])
                    if m >= NP:  # present-k: ship the bf16 tile directly
                        nc.scalar.dma_start(
                            kout_d[(m - NP) * P:(m - NP + 1) * P,
                                   h * 512:(h + 1) * 512],
                            dst[:, h * 512:(h + 1) * 512])

                def v_unit(i, ps1):
                    pu = ps1.tile([P, FPC], F32, tag="ps1", name="ps1")
                    for kk in range(KT):
                        nc.tensor.matmul(
                            pu[:],
                            xT_sb[:, kk, i * P:(i + 1) * P],
                            wv_sb[:, kk, :],
                            start=(kk == 0), stop=(kk == KT - 1),
                        )
                    nc.vector.tensor_copy(
                        v_sb[i][:].rearrange("p h d -> p (h d)"), pu[:])
                    nc.scalar.dma_start(
                        vout_d[i * P:(i + 1) * P, :],
                        v_sb[i][:].rearrange("p h d -> p (h d)"))

                def sc_unit(args, _ps1=None):
                    p, c, j = args
                    pT = pT_tiles[p]
                    partial = j * P >= c * 512
                    y0 = j * P - c * 512 if partial else 0
                    for hs in range(2):
                        pw = ps_w.tile([P, 512], F32, tag="psw", name="psw")
                        nc.tensor.matmul(
                            pw[:, y0:512],
                            kT_sb[p][hs * 64:(hs + 1) * 64, j * P:(j + 1) * P],
                            qT_sb[p][hs * 64:(hs + 1) * 64,
                                     c * 512 + y0:(c + 1) * 512],
                            start=True, stop=True,
                        )
                        nc.scalar.activation(
                            pT[:, hs, j, c * 512 + y0:(c + 1) * 512],
                            pw[:, y0:512], Exp, scale=SCALE)
                        if partial:
                            # causal-zero the diagonal 128-col strip
                            diag = pT[:, hs, j, j * P:(j + 1) * P]
                            nc.gpsimd.affine_select(
                                out=diag, in_=diag,
                                compare_op=mybir.AluOpType.is_ge,
                                fill=0.0, base=0,
                                pattern=[[1, P]],
                                channel_multiplier=-1,
                            )

                def pv_unit(args, pools):
                    ps_a, ps_s = pools
                    p, c, hs = args
                    pT = pT_tiles[p]
                    h = 2 * p + hs
                    pa = ps_a.tile([HD, 512], F32, tag="psa", name="psa")
                    ps2 = ps_s.tile([HD, 512], F32, tag="ps2", name="ps2")
                    js = list(_valid_j(c))
                    for j in js:
                        y0 = max(0, j * P - c * 512)
                        rhs = pT[:, hs, j, c * 512 + y0:(c + 1) * 512]
                        nc.tensor.matmul(
                            pa[:, y0:512], v_sb[j][:, h, :], rhs,
                            start=(j == js[0]), stop=(j == js[-1]))
                        nc.tensor.matmul(
                            ps2[:, y0:512], ones64[:], rhs,
                            start=(j == js[0]), stop=(j == js[-1]))
                    divb = small.tile([HD, 512], F32, tag="divb", name="divb")
                    nc.vector.reciprocal_approx_fast(out=divb[:], in_=ps2[:])
                    if hs == 0:
                        nc.vector.tensor_mul(
                            out=aT_sb[p][0:HD, c * 512:(c + 1) * 512],
                            in0=pa[:], in1=divb[:])
                    else:
                        tmp = small.tile([HD, 512], BF16, tag="tmp",
                                         name="tmp")
                        nc.vector.tensor_mul(out=tmp[:], in0=pa[:],
                                             in1=divb[:])
                        nc.sync.dma_start(
                            aT_sb[p][HD:P, c * 512:(c + 1) * 512], tmp[:])

                def sc_pair(p):
                    pT_tiles[p] = att.tile([P, 2, 8, S], BF16, tag="pT",
                                           name="pT")
                    return [(p, c, j) for c in range(2) for j in _valid_j(c)]

                def pv_pair(p):
                    return [(p, c, hs) for c in range(2) for hs in range(2)]

                # ---------------- schedule ----------------------------------
                with tc.tile_pool(name="ps_1", bufs=3, space="PSUM") as ps1:
                    # HAM warmup: dummy matmuls while input DMAs stream
                    warm = small.tile([P, 512], BF16, tag="warm", name="warm")
                    nc.gpsimd.memset(warm[:], 0.0)
                    pwarm = ps1.tile([P, 512], F32, tag="ps1", name="ps1")
                    for _ in range(28):
                        nc.tensor.matmul(pwarm[:], warm[:, 0:P], warm[:],
                                         start=True, stop=True)
                    qk = [(m, h) for m in (0, 4, 1, 5, 2, 6, 3, 7)
                          for h in range(2)]
                    for u in qk[:8]:
                        qk_unit(u, ps1)
                    for kind, u in _interleave(
                            [("sc", u) for u in sc_pair(0)],
                            [("qk", u) for u in qk[8:12]], 3):
                        (sc_unit if kind == "sc" else qk_unit)(u, ps1)
                    for kind, u in _interleave(
                            [("sc", u) for u in sc_pair(1)],
                            [("qk", u) for u in qk[12:16]], 3):
                        (sc_unit if kind == "sc" else qk_unit)(u, ps1)
                    for i in range(KT):
                        v_unit(i, ps1)

                with (
                    tc.tile_pool(name="ps_a", bufs=2, space="PSUM") as ps_a,
                    tc.tile_pool(name="ps_s", bufs=2, space="PSUM") as ps_s,
                    tc.tile_pool(name="ps_o", bufs=2, space="PSUM") as ps_o,
                ):
                    pools = (ps_a, ps_s)
                    for kind, u in _interleave(
                            [("sc", u) for u in sc_pair(2)],
                            [("pv", u) for u in pv_pair(0)], 3):
                        (sc_unit if kind == "sc" else pv_unit)(u, pools)
                    for kind, u in _interleave(
                            [("sc", u) for u in sc_pair(3)],
                            [("pv", u) for u in pv_pair(1)], 3):
                        (sc_unit if kind == "sc" else pv_unit)(u, pools)
                    for u in pv_pair(2):
                        pv_unit(u, pools)
                    for u in pv_pair(3):
                        pv_unit(u, pools)

                    # ---- c_proj -------------------------------------------
                    for i in range(KT):
                        for n in range(2):
                            po = ps_o.tile([P, 512], F32, tag="pso",
                                           name="pso")
                            for m in range(NP):
                                nc.tensor.matmul(
                                    po[:],
                                    aT_sb[m][:, i * P:(i + 1) * P],
                                    wp_sb[:, m, n * 512:(n + 1) * 512],
                                    start=(m == 0),
                                    stop=(m == NP - 1),
                                )
                            ost = stage.tile([P, 512], F32, tag="ostage",
                                             name="ostage")
                            nc.vector.tensor_copy(ost[:], po[:])
                            nc.sync.dma_start(
                                out_d[i * P:(i + 1) * P,
                                      n * 512:(n + 1) * 512], ost[:])

    nc.finalize()
    return nc


def _get_nc():
    global _CACHED_NC
    if _CACHED_NC is None:
        _CACHED_NC = _build_nc()
    return _CACHED_NC


def kernel(x, w_attn, b_attn, w_proj, b_proj, causal_bias):
    x = np.asarray(x, dtype=np.float32)
    w_attn = np.asarray(w_attn, dtype=np.float32)
    b_attn = np.asarray(b_attn, dtype=np.float32)
    w_proj = np.asarray(w_proj, dtype=np.float32)
    b_proj = np.asarray(b_proj, dtype=np.float32)

    nc = _get_nc()

    bf = ml_dtypes.bfloat16
    xT = [np.ascontiguousarray(x[b].T).astype(bf) for b in range(B)]
    wqk = []
    wv = []
    wp = []
    bqk = []
    for g in range(HG):
        qs = slice(g * FPC, (g + 1) * FPC)
        ks = slice(NX + g * FPC, NX + (g + 1) * FPC)
        vs = slice(2 * NX + g * FPC, 2 * NX + (g + 1) * FPC)
        wqk.append(np.ascontiguousarray(
            np.concatenate([w_attn[:, qs], w_attn[:, ks]], axis=1)).astype(bf))
        wv.append(np.ascontiguousarray(w_attn[:, vs]).astype(bf))
        wp.append(np.ascontiguousarray(
            w_proj[g * FPC:(g + 1) * FPC, :]).astype(bf))
        bqk.append(np.ascontiguousarray(
            np.concatenate([b_attn[qs], b_attn[ks]]).reshape(8, P).T
        ).astype(np.float32))

    in_maps = []
    for b in range(B):
        for g in range(HG):
            in_maps.append({
                "xT": xT[b], "w_qk": wqk[g], "w_v": wv[g], "w_p": wp[g],
                "b_qk": bqk[g],
            })

    global _last_in_maps
    _last_in_maps = in_maps
    res = run_bass_kernel_spmd(nc, in_maps, core_ids=list(range(8)))

    # ---- host-side gather / unshard ------------------------------------
    out = np.empty((B, S, NX), dtype=np.float32)
    present = np.empty((2, B, NH, S, HD), dtype=np.float32)

    # exact linear corrections for the v and proj biases (device omits them)
    corr = b_proj.astype(np.float64).copy()
    for g in range(HG):
        b_v = b_attn[2 * NX + g * FPC: 2 * NX + (g + 1) * FPC]
        corr += b_v.astype(np.float64) @ w_proj[
            g * FPC:(g + 1) * FPC, :].astype(np.float64)
    corr = corr.astype(np.float32)

    for b in range(B):
        r0 = res.results[b * HG + 0]
        r1 = res.results[b * HG + 1]
        out[b] = r0["out_p"] + r1["out_p"] + corr[None, :]
        for g, r in ((0, r0), (1, r1)):
            b_v = b_attn[2 * NX + g * FPC: 2 * NX + (g + 1) * FPC]
            # k_out already includes the (q,k) bias; v bias added here
            k_full = r["k_out"].astype(np.float32)          # [FPC, S]
            v_full = r["v_out"].astype(np.float32) + b_v[None, :]  # [S, FPC]
            present[0, b, g * HPC:(g + 1) * HPC] = (
                k_full.reshape(HPC, HD, S).transpose(0, 2, 1))
            present[1, b, g * HPC:(g + 1) * HPC] = (
                v_full.reshape(S, HPC, HD).transpose(1, 0, 2))

    return out, present
